# revision 5
# baseline (speedup 1.0000x reference)
"""Trainium2 Bass kernel for nn_ExampleTiedDropout (scatter_memory).

reference: out = X * mask[:, :, None] where mask[b] has the first
int(0.2*S)=204 positions fixed to 1 and the remaining 820 positions
Bernoulli(0.1) keyed by fold_in(key(0), idx[b]).

Since the mask is exactly {0, 1}, the output is a row-sparse copy of X:
~28% of the (b, s) rows are copied verbatim, the rest are zero. The
kernel therefore:
  1. computes the mask on host with the same jax ops as the reference
     (bit-exact: same env -> same rbg backend bitstream),
  2. shards the batch data-parallel across 8 NeuronCores (4 examples,
     i.e. 4096 rows of 2048 floats, per core),
  3. on each core, gathers the kept rows HBM->SBUF and scatters them
     back to the output with indirect DMAs driven by a host-built row
     index table (padded entries are out-of-bounds and skipped),
  4. leaves dropped rows untouched -- ExternalOutput buffers are
     donated zero-filled buffers, so unwritten rows read back as 0.
This moves only ~2 x 9.4 MB per core instead of 2 x 33.5 MB dense.
"""
import numpy as np

B, S, H = 32, 1024, 2048
N_CORES = 8
BPC = B // N_CORES           # examples per core
ROWS = BPC * S               # rows of H floats per core
P = 128                      # SBUF partitions
P_FIXED, P_MEM, MASK_SEED = 0.2, 0.1, 0
OOB_IDX = 1 << 24            # padded index; > bounds_check -> DMA skips it

_PROGRAM_CACHE = {}
LAST_RESULTS = None


def _ensure_ntff_hook():
    """The concourse trace path imports antenv.axon_hooks, which this image's
    antenv package lacks -- a hard crash when tracing is requested. Provide
    the missing module and register the boot's ctypes-based hook so NTFF
    profiling works as designed. No-op when the real module exists."""
    try:
        import antenv.axon_hooks  # noqa: F401
        return
    except ImportError:
        pass
    import sys
    import types

    mod = types.ModuleType("antenv.axon_hooks")
    mod._hook = None
    mod.set_axon_ntff_profile_hook = lambda h: setattr(mod, "_hook", h)
    mod.get_axon_ntff_profile_hook = lambda: mod._hook
    sys.modules["antenv.axon_hooks"] = mod
    try:
        import antenv
        antenv.axon_hooks = mod
    except ImportError:
        pass
    try:
        from trn_agent_boot.trn_boot import _ntff_profile_via_ctypes
        mod._hook = _ntff_profile_via_ctypes("/opt/axon/libaxon_pjrt.so")
    except Exception:
        pass  # hook stays None: concourse logs a warning and skips tracing


_ensure_ntff_hook()


def _tied_dropout_mask_host(idx_np):
    """Verbatim replica of reference._tied_dropout_mask, evaluated with the
    process-default jax backend/PRNG so the bits match the grader's
    reference run in the same environment."""
    import jax
    import jax.numpy as jnp

    n_fixed = int(P_FIXED * S)
    n_rand = S - n_fixed
    base = jax.random.key(MASK_SEED)

    def row_mask(i):
        k = jax.random.fold_in(base, i)
        return jax.random.bernoulli(k, P_MEM, (n_rand,)).astype(jnp.float32)

    idx = jnp.asarray(idx_np)
    rand_part = jax.vmap(row_mask)(idx)
    fixed_part = jnp.ones((idx.shape[0], n_fixed), jnp.float32)
    return np.asarray(jnp.concatenate([fixed_part, rand_part], axis=1))


N_FIXED = int(P_FIXED * S)   # 204 leading rows per example, always kept


def _build_program(n_tiles):
    """Static SPMD program per core:
      - one direct DRAM->DRAM HWDGE copy of the fixed prefix rows
        (x[e, :204, :] -> y[e, :204, :] for the 4 local examples),
      - n_tiles x (indirect gather 128 scattered rows -> SBUF,
        indirect scatter back to y) driven by the kidx input.
    Value-agnostic: row choices live in the kidx tensor; padded entries
    are out-of-bounds and skipped by the DMA engine."""
    import concourse.bacc as bacc
    import concourse.bass as bass
    import concourse.tile as tile
    from concourse import mybir

    nc = bacc.Bacc("TRN2", target_bir_lowering=False, debug=False,
                   num_devices=N_CORES)
    x = nc.dram_tensor("x", [ROWS, H], mybir.dt.float32, kind="ExternalInput")
    kidx = nc.dram_tensor("kidx", [P, max(n_tiles, 1)], mybir.dt.int32,
                          kind="ExternalInput")
    y = nc.dram_tensor("y", [ROWS, H], mybir.dt.float32, kind="ExternalOutput")

    x4 = x.rearrange("(e s) h -> e s h", s=S)
    y4 = y.rearrange("(e s) h -> e s h", s=S)

    with tile.TileContext(nc) as tc:
        with (
            tc.tile_pool(name="xp", bufs=max(n_tiles, 1)) as xp,
            tc.tile_pool(name="ip", bufs=1) as ip,
        ):
            # Fixed prefix: bulk contiguous copy, no SBUF bounce, no index
            # dependency -- starts immediately on the SP HWDGE ring.
            nc.sync.dma_start(out=y4[:, :N_FIXED, :], in_=x4[:, :N_FIXED, :])

            if n_tiles > 0:
                # One small load for the whole index table (ACT HWDGE ring).
                it = ip.tile([P, n_tiles], mybir.dt.int32)
                nc.scalar.dma_start(out=it[:], in_=kidx[:])
                for t in range(n_tiles):
                    xt = xp.tile([P, H], mybir.dt.float32)
                    nc.gpsimd.indirect_dma_start(
                        out=xt[:],
                        out_offset=None,
                        in_=x[:],
                        in_offset=bass.IndirectOffsetOnAxis(
                            ap=it[:, t:t + 1], axis=0),
                        bounds_check=ROWS - 1,
                        oob_is_err=False,
                    )
                    nc.gpsimd.indirect_dma_start(
                        out=y[:],
                        out_offset=bass.IndirectOffsetOnAxis(
                            ap=it[:, t:t + 1], axis=0),
                        in_=xt[:],
                        in_offset=None,
                        bounds_check=ROWS - 1,
                        oob_is_err=False,
                    )
    nc.compile()
    return nc


def kernel(X, idx):
    global LAST_RESULTS
    from concourse.bass_utils import run_bass_kernel_spmd

    X = np.ascontiguousarray(np.asarray(X, dtype=np.float32))
    idx = np.asarray(idx, dtype=np.int32)

    mask = _tied_dropout_mask_host(idx)          # [B, S] float32 of {0,1}
    keep = mask.reshape(N_CORES, ROWS) > 0.5     # [8, 4096] bool
    # The fixed prefix rows (s < N_FIXED of each example) are copied by the
    # static bulk DMA; only scattered kept rows go through the index table.
    keep[:, :] &= np.tile(np.arange(S) >= N_FIXED, BPC)[None, :]

    keep_rows = [np.flatnonzero(keep[c]).astype(np.int32) for c in range(N_CORES)]
    max_keep = max(len(r) for r in keep_rows)
    n_tiles = -(-max_keep // P)                  # same static tile count per core

    in_maps = []
    for c in range(N_CORES):
        flat = np.full((max(n_tiles, 1) * P,), OOB_IDX, dtype=np.int32)
        r = keep_rows[c]
        flat[: len(r)] = r
        # tile t reads column t -> kidx[p, t] = flat[t*P + p]
        kidx = np.ascontiguousarray(flat.reshape(max(n_tiles, 1), P).T)
        in_maps.append({
            "x": X[c * BPC:(c + 1) * BPC].reshape(ROWS, H),
            "kidx": kidx,
        })

    if n_tiles not in _PROGRAM_CACHE:
        _PROGRAM_CACHE[n_tiles] = _build_program(n_tiles)
    nc = _PROGRAM_CACHE[n_tiles]

    res = run_bass_kernel_spmd(nc, in_maps, list(range(N_CORES)))
    LAST_RESULTS = res

    out = np.empty((B, S, H), dtype=np.float32)
    for c in range(N_CORES):
        out[c * BPC:(c + 1) * BPC] = res.results[c]["y"].reshape(BPC, S, H)
    return out


# revision 8
# speedup vs baseline: 1.4153x; 1.4153x over previous
"""Trainium2 Bass kernel for nn_ExampleTiedDropout (scatter_memory).

reference: out = X * mask[:, :, None] where mask[b] has the first
int(0.2*S)=204 positions fixed to 1 and the remaining 820 positions
Bernoulli(0.1) keyed by fold_in(key(0), idx[b]).

Since the mask is exactly {0, 1}, the output is a row-sparse copy of X:
~28% of the (b, s) rows are copied verbatim, the rest are zero. The
kernel therefore:
  1. computes the mask on host with the same jax ops as the reference
     (bit-exact: same env -> same rbg backend bitstream),
  2. shards the batch data-parallel across 8 NeuronCores (4 examples,
     i.e. 4096 rows of 2048 floats, per core),
  3. on each core, gathers the kept rows HBM->SBUF and scatters them
     back to the output with indirect DMAs driven by a host-built row
     index table (padded entries are out-of-bounds and skipped),
  4. leaves dropped rows untouched -- ExternalOutput buffers are
     donated zero-filled buffers, so unwritten rows read back as 0.
This moves only ~2 x 9.4 MB per core instead of 2 x 33.5 MB dense.
"""
import numpy as np

B, S, H = 32, 1024, 2048
N_CORES = 8
BPC = B // N_CORES           # examples per core
ROWS = BPC * S               # rows of H floats per core
P = 128                      # SBUF partitions
P_FIXED, P_MEM, MASK_SEED = 0.2, 0.1, 0
OOB_IDX = 1 << 24            # padded index; > bounds_check -> DMA skips it

_PROGRAM_CACHE = {}
LAST_RESULTS = None


def _ensure_ntff_hook():
    """The concourse trace path imports antenv.axon_hooks, which this image's
    antenv package lacks -- a hard crash when tracing is requested. Provide
    the missing module and register the boot's ctypes-based hook so NTFF
    profiling works as designed. No-op when the real module exists."""
    try:
        import antenv.axon_hooks  # noqa: F401
        return
    except ImportError:
        pass
    import sys
    import types

    mod = types.ModuleType("antenv.axon_hooks")
    mod._hook = None
    mod.set_axon_ntff_profile_hook = lambda h: setattr(mod, "_hook", h)
    mod.get_axon_ntff_profile_hook = lambda: mod._hook
    sys.modules["antenv.axon_hooks"] = mod
    try:
        import antenv
        antenv.axon_hooks = mod
    except ImportError:
        pass
    try:
        from trn_agent_boot.trn_boot import _ntff_profile_via_ctypes
        mod._hook = _ntff_profile_via_ctypes("/opt/axon/libaxon_pjrt.so")
    except Exception:
        pass  # hook stays None: concourse logs a warning and skips tracing


_ensure_ntff_hook()


def _tied_dropout_mask_host(idx_np):
    """Verbatim replica of reference._tied_dropout_mask, evaluated with the
    process-default jax backend/PRNG so the bits match the grader's
    reference run in the same environment."""
    import jax
    import jax.numpy as jnp

    n_fixed = int(P_FIXED * S)
    n_rand = S - n_fixed
    base = jax.random.key(MASK_SEED)

    def row_mask(i):
        k = jax.random.fold_in(base, i)
        return jax.random.bernoulli(k, P_MEM, (n_rand,)).astype(jnp.float32)

    idx = jnp.asarray(idx_np)
    rand_part = jax.vmap(row_mask)(idx)
    fixed_part = jnp.ones((idx.shape[0], n_fixed), jnp.float32)
    return np.asarray(jnp.concatenate([fixed_part, rand_part], axis=1))


N_FIXED = int(P_FIXED * S)   # 204 leading rows per example, always kept


def _build_program(n_tiles):
    """Static SPMD program per core:
      - one direct DRAM->DRAM HWDGE copy of the fixed prefix rows
        (x[e, :204, :] -> y[e, :204, :] for the 4 local examples),
      - n_tiles x (indirect gather 128 scattered rows -> SBUF,
        indirect scatter back to y) driven by the kidx input.
    Value-agnostic: row choices live in the kidx tensor; padded entries
    are out-of-bounds and skipped by the DMA engine."""
    import concourse.bacc as bacc
    import concourse.bass as bass
    import concourse.tile as tile
    from concourse import mybir

    nc = bacc.Bacc("TRN2", target_bir_lowering=False, debug=False,
                   num_devices=N_CORES)
    x = nc.dram_tensor("x", [ROWS, H], mybir.dt.float32, kind="ExternalInput")
    kidx = nc.dram_tensor("kidx", [P, max(n_tiles, 1)], mybir.dt.int32,
                          kind="ExternalInput")
    y = nc.dram_tensor("y", [ROWS, H], mybir.dt.float32, kind="ExternalOutput")

    # The fixed prefix of example e (rows [e*S, e*S + N_FIXED)) is one
    # contiguous block of N_FIXED*H floats; view it as [128, N_FIXED*H/128]
    # so the copy is partition-mapped and spreads over all 16 SDMA engines.
    # (A direct DRAM->DRAM copy measures ~100 GB/s: no partition split.)
    FW = N_FIXED * H // P  # 3264

    def fixed_view(ap, e):
        blk = ap[e * S:e * S + N_FIXED, :]          # [204, 2048] contiguous
        return blk.rearrange("s h -> (s h)").rearrange("(p f) -> p f", p=P)

    with tile.TileContext(nc) as tc:
        with (
            tc.tile_pool(name="xp", bufs=max(n_tiles, 1)) as xp,
            tc.tile_pool(name="fp", bufs=BPC) as fp,
            tc.tile_pool(name="ip", bufs=1) as ip,
        ):
            # Fixed prefix: contiguous bulk copy through SBUF. Loads on the
            # SP HWDGE ring, stores on the ACT ring -- one-directional each,
            # both concurrent with the SWDGE indirect stream below.
            for e in range(BPC):
                ft = fp.tile([P, FW], mybir.dt.float32)
                nc.sync.dma_start(out=ft[:], in_=fixed_view(x, e))
                nc.scalar.dma_start(out=fixed_view(y, e), in_=ft[:])

            if n_tiles > 0:
                # One small load for the whole index table (ACT HWDGE ring).
                it = ip.tile([P, n_tiles], mybir.dt.int32)
                nc.scalar.dma_start(out=it[:], in_=kidx[:])
                for t in range(n_tiles):
                    xt = xp.tile([P, H], mybir.dt.float32)
                    nc.gpsimd.indirect_dma_start(
                        out=xt[:],
                        out_offset=None,
                        in_=x[:],
                        in_offset=bass.IndirectOffsetOnAxis(
                            ap=it[:, t:t + 1], axis=0),
                        bounds_check=ROWS - 1,
                        oob_is_err=False,
                    )
                    nc.gpsimd.indirect_dma_start(
                        out=y[:],
                        out_offset=bass.IndirectOffsetOnAxis(
                            ap=it[:, t:t + 1], axis=0),
                        in_=xt[:],
                        in_offset=None,
                        bounds_check=ROWS - 1,
                        oob_is_err=False,
                    )
    nc.compile()
    return nc


def kernel(X, idx):
    global LAST_RESULTS
    from concourse.bass_utils import run_bass_kernel_spmd

    X = np.ascontiguousarray(np.asarray(X, dtype=np.float32))
    idx = np.asarray(idx, dtype=np.int32)

    mask = _tied_dropout_mask_host(idx)          # [B, S] float32 of {0,1}
    keep = mask.reshape(N_CORES, ROWS) > 0.5     # [8, 4096] bool
    # The fixed prefix rows (s < N_FIXED of each example) are copied by the
    # static bulk DMA; only scattered kept rows go through the index table.
    keep[:, :] &= np.tile(np.arange(S) >= N_FIXED, BPC)[None, :]

    keep_rows = [np.flatnonzero(keep[c]).astype(np.int32) for c in range(N_CORES)]
    max_keep = max(len(r) for r in keep_rows)
    n_tiles = -(-max_keep // P)                  # same static tile count per core

    in_maps = []
    for c in range(N_CORES):
        flat = np.full((max(n_tiles, 1) * P,), OOB_IDX, dtype=np.int32)
        r = keep_rows[c]
        flat[: len(r)] = r
        # tile t reads column t -> kidx[p, t] = flat[t*P + p]
        kidx = np.ascontiguousarray(flat.reshape(max(n_tiles, 1), P).T)
        in_maps.append({
            "x": X[c * BPC:(c + 1) * BPC].reshape(ROWS, H),
            "kidx": kidx,
        })

    if n_tiles not in _PROGRAM_CACHE:
        _PROGRAM_CACHE[n_tiles] = _build_program(n_tiles)
    nc = _PROGRAM_CACHE[n_tiles]

    res = run_bass_kernel_spmd(nc, in_maps, list(range(N_CORES)))
    LAST_RESULTS = res

    out = np.empty((B, S, H), dtype=np.float32)
    for c in range(N_CORES):
        out[c * BPC:(c + 1) * BPC] = res.results[c]["y"].reshape(BPC, S, H)
    return out
